# revision 10
# baseline (speedup 1.0000x reference)
"""Trainium2 Bass kernel for nn_CrossAttention (b=4, lq=lkv=2048, dq=1024, dkv=768, 4 heads).

Sharding: 8 cores = (batch b in 0..3) x (head-group g in 0..1); each core handles
one batch and 2 of the 4 heads (512 of the 1024 head dims).  All activations are
fed to the device pre-transposed ([model_dim, seq]) so every matmul contracts
over the partition dimension with zero on-device transposes:

  qhT  [512,2048] = WqT.T @ qT          (proj, contraction over dq=1024)
  khT  [512,2048] = WkT.T @ kvT         (proj, contraction over dkv=768)
  vh   [2048,512] = kvT_chunk.T @ WvT   (proj, natural layout)
  sT   [2048,2048] per head = khT_h.T @ qhT_h    (scoresT: lkv on partitions)
  eT   = exp(sT / 16)                   (no max-subtraction needed: |s| <~ 6)
  ctxT [256,2048] per head = vh_h.T(as lhsT) ... accumulated over lkv tiles
  sum  [1,2048]  per head = ones.T @ eT (softmax denominator via M=1 matmul)
  ctxT normalized by broadcast(1/sum) (K=1 ones matmul broadcast + DVE mul)
  outT [1024,2048] = WoT.T @ ctxT       (output proj over the core's 512 dims)

Host gathers: out[b] = (outT[core 2b] + outT[core 2b+1]).T + bo.
"""

import numpy as np

B = 4
LQ = 2048
LKV = 2048
DQ = 1024
DKV = 768
HD = 256  # per-head dim
GH = 512  # head dims per core (2 heads)
P = 128
NCORES = 8
NQ = LQ // 512  # lq chunks of 512
KT_Q = DQ // P  # 8
KT_KV = DKV // P  # 6
KT_L = LKV // P  # 16

USE_F32R = True  # full-rate fp32 matmuls (vs 4 cyc/row for plain fp32)
TRACE = False

_COMPILED = None
last_exec_time_ns = None
last_profile = None


def _emit(tc, aps):
    from contextlib import ExitStack

    import concourse.bass as bass
    import concourse.mybir as mybir

    nc = tc.nc
    f32 = mybir.dt.float32
    mm_dt = mybir.dt.float32r if USE_F32R else mybir.dt.float32
    Exp = mybir.ActivationFunctionType.Exp

    def mc(ap):  # operands are already typed with the matmul dtype
        return ap

    qT, kvT, WqT, WkT, WvT, WoT, outT = (
        aps["qT"], aps["kvT"], aps["WqT"], aps["WkT"], aps["WvT"], aps["WoT"],
        aps["outT"],
    )

    with ExitStack() as top:
        # persistent SBUF tensors
        khT_pool = top.enter_context(tc.tile_pool(name="khT", bufs=1))
        qhT_pool = top.enter_context(tc.tile_pool(name="qhT", bufs=1))
        vh_pool = top.enter_context(tc.tile_pool(name="vh", bufs=1))
        const_pool = top.enter_context(tc.tile_pool(name="const", bufs=1))

        khT = [khT_pool.tile([P, LKV], mm_dt, tag=f"khT{i}", name=f"khT{i}") for i in range(4)]
        qhT = [qhT_pool.tile([P, LQ], mm_dt, tag=f"qhT{i}", name=f"qhT{i}") for i in range(4)]
        vh = [vh_pool.tile([P, GH], mm_dt, tag=f"vh{i}", name=f"vh{i}") for i in range(KT_L)]

        ones_col = const_pool.tile([P, 1], mm_dt, tag="ones_col", name="ones_col")
        ones_row = const_pool.tile([1, P], mm_dt, tag="ones_row", name="ones_row")
        ones_f32 = const_pool.tile([P, 1], f32, tag="ones_f32", name="ones_f32")
        ones_f32r = const_pool.tile([1, P], f32, tag="ones_f32r", name="ones_f32r")
        nc.vector.memset(ones_f32[:], 1.0)
        nc.vector.memset(ones_f32r[:], 1.0)
        nc.vector.tensor_copy(ones_col[:], ones_f32[:])
        nc.vector.tensor_copy(ones_row[:], ones_f32r[:])

        # ---------------- Phase A1: khT and vh from kvT ----------------
        with ExitStack() as ph:
            kv_pool = ph.enter_context(tc.tile_pool(name="kv", bufs=1))
            wk_pool = ph.enter_context(tc.tile_pool(name="wk", bufs=1))
            wv_pool = ph.enter_context(tc.tile_pool(name="wv", bufs=1))
            psA = ph.enter_context(tc.tile_pool(name="psA", bufs=4, space="PSUM"))

            kv_t, wk_t, wv_t = [], [], []
            for kt in range(KT_KV):
                t = kv_pool.tile([P, LKV], mm_dt, tag=f"kv{kt}", name=f"kv{kt}")
                nc.sync.dma_start(t[:], kvT[kt * P:(kt + 1) * P, :])
                kv_t.append(t)
                w = wk_pool.tile([P, GH], mm_dt, tag=f"wk{kt}", name=f"wk{kt}")
                nc.sync.dma_start(w[:], WkT[kt * P:(kt + 1) * P, :])
                wk_t.append(w)
                w = wv_pool.tile([P, GH], mm_dt, tag=f"wv{kt}", name=f"wv{kt}")
                nc.sync.dma_start(w[:], WvT[kt * P:(kt + 1) * P, :])
                wv_t.append(w)

            for m in range(4):  # head-dim tiles of the core's 512
                for n in range(4):  # lkv chunks of 512
                    ps = psA.tile([P, 512], f32, tag="psA", name="psA")
                    for kt in range(KT_KV):
                        nc.tensor.matmul(
                            ps[:],
                            lhsT=mc(wk_t[kt][:, m * P:(m + 1) * P]),
                            rhs=mc(kv_t[kt][:, n * 512:(n + 1) * 512]),
                            start=(kt == 0),
                            stop=(kt == KT_KV - 1),
                        )
                    nc.vector.tensor_copy(khT[m][:, n * 512:(n + 1) * 512], ps[:])

            for l in range(KT_L):  # lkv tiles of 128
                ps = psA.tile([P, 512], f32, tag="psA", name="psA")
                for kt in range(KT_KV):
                    nc.tensor.matmul(
                        ps[:],
                        lhsT=mc(kv_t[kt][:, l * P:(l + 1) * P]),
                        rhs=mc(wv_t[kt][:]),
                        start=(kt == 0),
                        stop=(kt == KT_KV - 1),
                    )
                nc.vector.tensor_copy(vh[l][:], ps[:])

        # ---------------- Phase A2: qhT from qT ----------------
        with ExitStack() as ph:
            q_pool = ph.enter_context(tc.tile_pool(name="q", bufs=2))
            wq_pool = ph.enter_context(tc.tile_pool(name="wq", bufs=1))
            psA2 = ph.enter_context(tc.tile_pool(name="psA2", bufs=4, space="PSUM"))

            wq_t = []
            for kt in range(KT_Q):
                w = wq_pool.tile([P, GH], mm_dt, tag=f"wq{kt}", name=f"wq{kt}")
                nc.sync.dma_start(w[:], WqT[kt * P:(kt + 1) * P, :])
                wq_t.append(w)

            for n in range(NQ):
                qn_t = []
                for kt in range(KT_Q):
                    t = q_pool.tile([P, 512], mm_dt, tag=f"q{kt}", name=f"q{kt}")
                    nc.sync.dma_start(
                        t[:], qT[kt * P:(kt + 1) * P, n * 512:(n + 1) * 512]
                    )
                    qn_t.append(t)
                for m in range(4):
                    ps = psA2.tile([P, 512], f32, tag="psA2", name="psA2")
                    for kt in range(KT_Q):
                        nc.tensor.matmul(
                            ps[:],
                            lhsT=mc(wq_t[kt][:, m * P:(m + 1) * P]),
                            rhs=mc(qn_t[kt][:]),
                            start=(kt == 0),
                            stop=(kt == KT_Q - 1),
                        )
                    nc.vector.tensor_copy(qhT[m][:, n * 512:(n + 1) * 512], ps[:])

        # ---------------- Phases B+C ----------------
        bc_top = top.enter_context(ExitStack())
        ctxT_pool = bc_top.enter_context(tc.tile_pool(name="ctxT", bufs=1))
        ctxT = [ctxT_pool.tile([P, LQ], mm_dt, tag=f"ctxT{i}", name=f"ctxT{i}")
                for i in range(4)]

        # ---------------- Phase B: attention per head ----------------
        with ExitStack() as ph:
            ps_s = ph.enter_context(tc.tile_pool(name="ps_s", bufs=3, space="PSUM"))
            ps_ctx = ph.enter_context(tc.tile_pool(name="ps_ctx", bufs=2, space="PSUM"))
            ps_sum = ph.enter_context(tc.tile_pool(name="ps_sum", bufs=2, space="PSUM"))
            ps_b = ph.enter_context(tc.tile_pool(name="ps_b", bufs=1, space="PSUM"))
            et_pool = ph.enter_context(tc.tile_pool(name="et", bufs=6))
            small = ph.enter_context(tc.tile_pool(name="small", bufs=2))
            bc_pool = ph.enter_context(tc.tile_pool(name="bc", bufs=2))

            scale = 1.0 / np.sqrt(HD)

            for h in range(2):
                k0, k1 = khT[2 * h], khT[2 * h + 1]
                q0, q1 = qhT[2 * h], qhT[2 * h + 1]
                for n in range(NQ):
                    nsl = slice(n * 512, (n + 1) * 512)
                    pc0 = ps_ctx.tile([P, 512], f32, tag="pc", name="pc")
                    pc1 = ps_ctx.tile([P, 512], f32, tag="pc", name="pc")
                    psm = ps_sum.tile([1, 512], f32, tag="psm", name="psm")

                    et_prev = None
                    for kt in range(KT_L):
                        ksl = slice(kt * P, (kt + 1) * P)
                        ps = ps_s.tile([P, 512], f32, tag="ps_s", name="ps_s")
                        nc.tensor.matmul(
                            ps[:], lhsT=mc(k0[:, ksl]), rhs=mc(q0[:, nsl]),
                            start=True, stop=False,
                        )
                        nc.tensor.matmul(
                            ps[:], lhsT=mc(k1[:, ksl]), rhs=mc(q1[:, nsl]),
                            start=False, stop=True,
                        )
                        et = et_pool.tile([P, 512], mm_dt, tag="et", name="et")
                        nc.scalar.activation(et[:], ps[:], Exp, scale=scale)

                        if et_prev is not None:
                            pkt, pet = et_prev
                            _pv(nc, mc, vh, pkt, h, pet, pc0, pc1, psm, ones_col,
                                first=(pkt == 0), last=False)
                        et_prev = (kt, et)

                    pkt, pet = et_prev
                    _pv(nc, mc, vh, pkt, h, pet, pc0, pc1, psm, ones_col,
                        first=False, last=True)

                    # normalize: ctxT[:, nsl] = pc / broadcast(sum)
                    rc = small.tile([1, 512], mm_dt, tag="rc", name="rc")
                    with nc.allow_low_precision(reason="f32r softmax recip"):
                        nc.vector.reciprocal(rc[:], psm[:])
                    pb = ps_b.tile([P, 512], f32, tag="pb", name="pb")
                    nc.tensor.matmul(
                        pb[:], lhsT=mc(ones_row[:]), rhs=mc(rc[:]),
                        start=True, stop=True,
                    )
                    bc = bc_pool.tile([P, 512], f32, tag="bc", name="bc")
                    nc.vector.tensor_copy(bc[:], pb[:])
                    nc.vector.tensor_mul(ctxT[2 * h][:, nsl], pc0[:], bc[:])
                    nc.vector.tensor_mul(ctxT[2 * h + 1][:, nsl], pc1[:], bc[:])

        # ---------------- Phase C: output projection ----------------
        with ExitStack() as ph:
            wo_pool = ph.enter_context(tc.tile_pool(name="wo", bufs=1))
            psC = ph.enter_context(tc.tile_pool(name="psC", bufs=4, space="PSUM"))
            outC = ph.enter_context(tc.tile_pool(name="outC", bufs=4))

            wo_t = []
            for kt in range(4):
                w = wo_pool.tile([P, DQ], mm_dt, tag=f"wo{kt}", name=f"wo{kt}")
                nc.sync.dma_start(w[:], WoT[kt * P:(kt + 1) * P, :])
                wo_t.append(w)

            for m in range(DQ // P):  # 8
                for n in range(NQ):  # 4
                    ps = psC.tile([P, 512], f32, tag="psC", name="psC")
                    for kt in range(4):
                        nc.tensor.matmul(
                            ps[:],
                            lhsT=mc(wo_t[kt][:, m * P:(m + 1) * P]),
                            rhs=mc(ctxT[kt][:, n * 512:(n + 1) * 512]),
                            start=(kt == 0),
                            stop=(kt == 3),
                        )
                    ot = outC.tile([P, 512], f32, tag="ot", name="ot")
                    nc.scalar.copy(ot[:], ps[:])
                    nc.sync.dma_start(
                        outT[m * P:(m + 1) * P, n * 512:(n + 1) * 512], ot[:]
                    )


def _pv(nc, mc, vh, kt, h, et, pc0, pc1, psm, ones_col, first, last):
    hsl0 = slice(HD * h, HD * h + P)
    hsl1 = slice(HD * h + P, HD * h + 2 * P)
    nc.tensor.matmul(pc0[:], lhsT=mc(vh[kt][:, hsl0]), rhs=mc(et[:]),
                     start=first, stop=last)
    nc.tensor.matmul(pc1[:], lhsT=mc(vh[kt][:, hsl1]), rhs=mc(et[:]),
                     start=first, stop=last)
    nc.tensor.matmul(psm[:], lhsT=mc(ones_col[:]), rhs=mc(et[:]),
                     start=first, stop=last)


def _build():
    import concourse.bacc as bacc
    import concourse.mybir as mybir
    import concourse.tile as tile

    f32 = mybir.dt.float32
    in_dt = mybir.dt.float32r if USE_F32R else f32
    nc = bacc.Bacc("TRN2", target_bir_lowering=False, debug=False)
    aps = {
        "qT": nc.dram_tensor("qT", [DQ, LQ], in_dt, kind="ExternalInput").ap(),
        "kvT": nc.dram_tensor("kvT", [DKV, LKV], in_dt, kind="ExternalInput").ap(),
        "WqT": nc.dram_tensor("WqT", [DQ, GH], in_dt, kind="ExternalInput").ap(),
        "WkT": nc.dram_tensor("WkT", [DKV, GH], in_dt, kind="ExternalInput").ap(),
        "WvT": nc.dram_tensor("WvT", [DKV, GH], in_dt, kind="ExternalInput").ap(),
        "WoT": nc.dram_tensor("WoT", [GH, DQ], in_dt, kind="ExternalInput").ap(),
        "outT": nc.dram_tensor("outT", [DQ, LQ], f32, kind="ExternalOutput").ap(),
    }
    with tile.TileContext(nc) as tc:
        _emit(tc, aps)
    nc.compile()
    return nc


def make_in_maps(q, kv, Wq, Wk, Wv, Wo):
    in_maps = []
    for c in range(NCORES):
        b, g = divmod(c, 2)
        hs = slice(g * GH, (g + 1) * GH)
        in_maps.append({
            "qT": np.ascontiguousarray(q[b].T),
            "kvT": np.ascontiguousarray(kv[b].T),
            "WqT": np.ascontiguousarray(Wq[hs, :].T),
            "WkT": np.ascontiguousarray(Wk[hs, :].T),
            "WvT": np.ascontiguousarray(Wv[hs, :].T),
            "WoT": np.ascontiguousarray(Wo[:, hs].T),
        })
    return in_maps


def kernel(q, kv, Wq, Wk, Wv, Wo, bo):
    global _COMPILED, last_exec_time_ns, last_profile
    from concourse.bass_utils import run_bass_kernel_spmd

    if _COMPILED is None:
        _COMPILED = _build()
    nc = _COMPILED

    q = np.asarray(q, np.float32)
    kv = np.asarray(kv, np.float32)
    Wq = np.asarray(Wq, np.float32)
    Wk = np.asarray(Wk, np.float32)
    Wv = np.asarray(Wv, np.float32)
    Wo = np.asarray(Wo, np.float32)
    bo = np.asarray(bo, np.float32)

    in_maps = make_in_maps(q, kv, Wq, Wk, Wv, Wo)
    res = run_bass_kernel_spmd(nc, in_maps, core_ids=list(range(NCORES)),
                               trace=TRACE)
    last_exec_time_ns = res.exec_time_ns
    last_profile = res.profile_json

    out = np.empty((B, LQ, DQ), np.float32)
    for b in range(B):
        acc = res.results[2 * b]["outT"] + res.results[2 * b + 1]["outT"]
        out[b] = acc.T + bo
    return out


# revision 12
# speedup vs baseline: 1.2397x; 1.2397x over previous
"""Trainium2 Bass kernel for nn_CrossAttention (b=4, lq=lkv=2048, dq=1024, dkv=768, 4 heads).

Sharding: 8 cores = (batch b in 0..3) x (head-group g in 0..1); each core handles
one batch and 2 of the 4 heads (512 of the 1024 head dims).  All activations are
fed to the device pre-transposed ([model_dim, seq]) so every matmul contracts
over the partition dimension with zero on-device transposes:

  qhT  [512,2048] = WqT.T @ qT          (proj, contraction over dq=1024)
  khT  [512,2048] = WkT.T @ kvT         (proj, contraction over dkv=768)
  vh   [2048,512] = kvT_chunk.T @ WvT   (proj, natural layout)
  sT   [2048,2048] per head = khT_h.T @ qhT_h    (scoresT: lkv on partitions)
  eT   = exp(sT / 16)                   (no max-subtraction needed: |s| <~ 6)
  ctxT [256,2048] per head accumulated over lkv tiles (lhsT=vh, rhs=eT)
  sum  via DVE add-tree over eT tiles + one ones[128,128] matmul
        (every psum partition gets the column sum -> 128-lane reciprocal)
  ctxT normalized by DVE mul with the reciprocal tile; the normalization
        tail for chunk i is emitted inside chunk i+1 so PE never stalls
  outT [1024,2048] = WoT.T @ ctxT       (output proj over the core's 512 dims)

Matmuls run as float32r (full-rate fp32, TF32-ish rounding, ~4e-4 rel err).
Host gathers: out[b] = (outT[core 2b] + outT[core 2b+1]).T + bo.
"""

import numpy as np

B = 4
LQ = 2048
LKV = 2048
DQ = 1024
DKV = 768
HD = 256  # per-head dim
GH = 512  # head dims per core (2 heads)
P = 128
NCORES = 8
NQ = LQ // 512  # lq chunks of 512
KT_Q = DQ // P  # 8
KT_KV = DKV // P  # 6
KT_L = LKV // P  # 16

USE_F32R = True  # full-rate fp32 matmuls (vs 4 cyc/row for plain fp32)
TRACE = False

_COMPILED = None
last_exec_time_ns = None
last_profile = None


def _emit(tc, aps):
    from contextlib import ExitStack

    import concourse.mybir as mybir

    nc = tc.nc
    f32 = mybir.dt.float32
    mm_dt = mybir.dt.float32r if USE_F32R else f32
    Exp = mybir.ActivationFunctionType.Exp

    qT, kvT, WqT, WkT, WvT, WoT, outT = (
        aps["qT"], aps["kvT"], aps["WqT"], aps["WkT"], aps["WvT"], aps["WoT"],
        aps["outT"],
    )
    kvT_r = kvT.rearrange("(k p) n -> p k n", p=P)  # [128, 6, 2048]
    qT_r = qT.rearrange("(k p) n -> p k n", p=P)    # [128, 8, 2048]
    WkT_r = WkT.rearrange("(k p) g -> p k g", p=P)  # [128, 6, 512]
    WvT_r = WvT.rearrange("(k p) g -> p k g", p=P)
    WqT_r = WqT.rearrange("(k p) g -> p k g", p=P)  # [128, 8, 512]
    WoT_r = WoT.rearrange("(k p) d -> p k d", p=P)  # [128, 4, 1024]

    with ExitStack() as top:
        # persistent SBUF tensors
        khT_pool = top.enter_context(tc.tile_pool(name="khT", bufs=1))
        qhT_pool = top.enter_context(tc.tile_pool(name="qhT", bufs=1))
        vh_pool = top.enter_context(tc.tile_pool(name="vh", bufs=1))
        const_pool = top.enter_context(tc.tile_pool(name="const", bufs=1))

        khT = [khT_pool.tile([P, LKV], mm_dt, tag=f"khT{i}", name=f"khT{i}")
               for i in range(4)]
        qhT = [qhT_pool.tile([P, LQ], mm_dt, tag=f"qhT{i}", name=f"qhT{i}")
               for i in range(4)]
        vh = [vh_pool.tile([P, GH], mm_dt, tag=f"vh{i}", name=f"vh{i}")
              for i in range(KT_L)]

        ones_sq = const_pool.tile([P, P], mm_dt, tag="ones_sq", name="ones_sq")
        ones_f32 = const_pool.tile([P, P], f32, tag="ones_f32", name="ones_f32")
        nc.vector.memset(ones_f32[:], 1.0)
        nc.vector.tensor_copy(ones_sq[:], ones_f32[:])

        # ---------------- Phase A: projections ----------------
        with ExitStack() as ph:
            w_pool = ph.enter_context(tc.tile_pool(name="w", bufs=1))
            kvc_pool = ph.enter_context(tc.tile_pool(name="kvc", bufs=2))
            qc_pool = ph.enter_context(tc.tile_pool(name="qc", bufs=2))
            psA = ph.enter_context(tc.tile_pool(name="psA", bufs=4, space="PSUM"))

            wk_t = w_pool.tile([P, KT_KV, GH], mm_dt, tag="wk", name="wk")
            wv_t = w_pool.tile([P, KT_KV, GH], mm_dt, tag="wv", name="wv")
            wq_t = w_pool.tile([P, KT_Q, GH], mm_dt, tag="wq", name="wq")
            nc.sync.dma_start(wk_t[:], WkT_r[:])
            kvc0 = kvc_pool.tile([P, KT_KV, 512], mm_dt, tag="kvc", name="kvc")
            nc.sync.dma_start(kvc0[:], kvT_r[:, :, 0:512])
            nc.sync.dma_start(wv_t[:], WvT_r[:])
            nc.sync.dma_start(wq_t[:], WqT_r[:])

            kvc_tiles = {0: kvc0}
            qc_tiles = {}

            def load_kvc(n):
                if n in kvc_tiles or n >= NQ:
                    return
                t = kvc_pool.tile([P, KT_KV, 512], mm_dt, tag="kvc", name="kvc")
                nc.sync.dma_start(t[:], kvT_r[:, :, n * 512:(n + 1) * 512])
                kvc_tiles[n] = t

            def load_qc(n):
                if n in qc_tiles or n >= NQ:
                    return
                t = qc_pool.tile([P, KT_Q, 512], mm_dt, tag="qc", name="qc")
                nc.sync.dma_start(t[:], qT_r[:, :, n * 512:(n + 1) * 512])
                qc_tiles[n] = t

            load_qc(0)
            for n in range(NQ):
                nsl = slice(n * 512, (n + 1) * 512)
                kvc = kvc_tiles[n]
                qc = qc_tiles[n]
                # prefetch next chunk
                load_kvc(n + 1)
                load_qc(n + 1)

                for m in range(4):  # khT head-dim tiles
                    ps = psA.tile([P, 512], f32, tag="psA", name="psA")
                    for kt in range(KT_KV):
                        nc.tensor.matmul(
                            ps[:],
                            lhsT=wk_t[:, kt, m * P:(m + 1) * P],
                            rhs=kvc[:, kt, :],
                            start=(kt == 0),
                            stop=(kt == KT_KV - 1),
                        )
                    nc.vector.tensor_copy(khT[m][:, nsl], ps[:])

                for lj in range(4):  # vh lkv tiles within this chunk
                    l = 4 * n + lj
                    ps = psA.tile([P, 512], f32, tag="psA", name="psA")
                    for kt in range(KT_KV):
                        nc.tensor.matmul(
                            ps[:],
                            lhsT=kvc[:, kt, lj * P:(lj + 1) * P],
                            rhs=wv_t[:, kt, :],
                            start=(kt == 0),
                            stop=(kt == KT_KV - 1),
                        )
                    nc.vector.tensor_copy(vh[l][:], ps[:])

                for m in range(4):  # qhT head-dim tiles
                    ps = psA.tile([P, 512], f32, tag="psA", name="psA")
                    for kt in range(KT_Q):
                        nc.tensor.matmul(
                            ps[:],
                            lhsT=wq_t[:, kt, m * P:(m + 1) * P],
                            rhs=qc[:, kt, :],
                            start=(kt == 0),
                            stop=(kt == KT_Q - 1),
                        )
                    nc.vector.tensor_copy(qhT[m][:, nsl], ps[:])

        # ---------------- Phases B+C ----------------
        bc_top = top.enter_context(ExitStack())
        ctxT_pool = bc_top.enter_context(tc.tile_pool(name="ctxT", bufs=1))
        ctxT = [ctxT_pool.tile([P, LQ], mm_dt, tag=f"ctxT{i}", name=f"ctxT{i}")
                for i in range(4)]

        # ---------------- Phase B: attention per head ----------------
        with ExitStack() as ph:
            ps_s = ph.enter_context(tc.tile_pool(name="ps_s", bufs=2, space="PSUM"))
            ps_ctx = ph.enter_context(tc.tile_pool(name="ps_ctx", bufs=4, space="PSUM"))
            ps_sum = ph.enter_context(tc.tile_pool(name="ps_sum", bufs=2, space="PSUM"))
            et_pool = ph.enter_context(tc.tile_pool(name="et", bufs=6))
            g_pool = ph.enter_context(tc.tile_pool(name="g", bufs=2))
            acc_pool = ph.enter_context(tc.tile_pool(name="acc", bufs=2))
            rcb_pool = ph.enter_context(tc.tile_pool(name="rcb", bufs=2))

            scale = 1.0 / np.sqrt(HD)
            pending_tail = [None]

            def flush_tail():
                if pending_tail[0] is not None:
                    pending_tail[0]()
                    pending_tail[0] = None

            for h in range(2):
                k0, k1 = khT[2 * h], khT[2 * h + 1]
                q0, q1 = qhT[2 * h], qhT[2 * h + 1]
                hsl0 = slice(HD * h, HD * h + P)
                hsl1 = slice(HD * h + P, HD * h + 2 * P)
                for n in range(NQ):
                    nsl = slice(n * 512, (n + 1) * 512)
                    pc0 = ps_ctx.tile([P, 512], f32, tag="pc", name="pc")
                    pc1 = ps_ctx.tile([P, 512], f32, tag="pc", name="pc")
                    g = [None] * 4

                    et_prev = None
                    for kt in range(KT_L):
                        ksl = slice(kt * P, (kt + 1) * P)
                        ps = ps_s.tile([P, 512], f32, tag="ps_s", name="ps_s")
                        nc.tensor.matmul(
                            ps[:], lhsT=k0[:, ksl], rhs=q0[:, nsl],
                            start=True, stop=False,
                        )
                        nc.tensor.matmul(
                            ps[:], lhsT=k1[:, ksl], rhs=q1[:, nsl],
                            start=False, stop=True,
                        )
                        et = et_pool.tile([P, 512], mm_dt, tag="et", name="et")
                        nc.scalar.activation(et[:], ps[:], Exp, scale=scale)

                        # sumexp tree accumulation on DVE
                        j = kt // 4
                        if kt % 4 == 0:
                            g[j] = g_pool.tile([P, 512], f32, tag=f"g{j}",
                                               name=f"g{j}")
                            nc.vector.tensor_copy(g[j][:], et[:])
                        else:
                            nc.vector.tensor_add(g[j][:], g[j][:], et[:])

                        if kt == 2:
                            flush_tail()

                        if et_prev is not None:
                            pkt, pet = et_prev
                            nc.tensor.matmul(
                                pc0[:], lhsT=vh[pkt][:, hsl0], rhs=pet[:],
                                start=(pkt == 0), stop=False,
                            )
                            nc.tensor.matmul(
                                pc1[:], lhsT=vh[pkt][:, hsl1], rhs=pet[:],
                                start=(pkt == 0), stop=False,
                            )
                        et_prev = (kt, et)

                    pkt, pet = et_prev
                    nc.tensor.matmul(pc0[:], lhsT=vh[pkt][:, hsl0], rhs=pet[:],
                                     start=False, stop=True)
                    nc.tensor.matmul(pc1[:], lhsT=vh[pkt][:, hsl1], rhs=pet[:],
                                     start=False, stop=True)

                    # finish the tree: acc = (g0+g1) + (g2+g3), typed for matmul
                    g01 = g_pool.tile([P, 512], f32, tag="g01", name="g01")
                    nc.vector.tensor_add(g01[:], g[0][:], g[1][:])
                    g23 = g_pool.tile([P, 512], f32, tag="g23", name="g23")
                    nc.vector.tensor_add(g23[:], g[2][:], g[3][:])
                    acc = acc_pool.tile([P, 512], mm_dt, tag="acc", name="acc")
                    nc.vector.tensor_add(acc[:], g01[:], g23[:])

                    def make_tail(pc0=pc0, pc1=pc1, acc=acc, h=h, nsl=nsl):
                        def tail():
                            pss = ps_sum.tile([P, 512], f32, tag="pss",
                                              name="pss")
                            nc.tensor.matmul(pss[:], lhsT=ones_sq[:],
                                             rhs=acc[:], start=True, stop=True)
                            rcb = rcb_pool.tile([P, 512], f32, tag="rcb",
                                                name="rcb")
                            nc.vector.reciprocal(rcb[:], pss[:])
                            nc.vector.tensor_mul(ctxT[2 * h][:, nsl], pc0[:],
                                                 rcb[:])
                            nc.vector.tensor_mul(ctxT[2 * h + 1][:, nsl],
                                                 pc1[:], rcb[:])
                        return tail

                    pending_tail[0] = make_tail()
            flush_tail()

        # ---------------- Phase C: output projection ----------------
        with ExitStack() as ph:
            wo_pool = ph.enter_context(tc.tile_pool(name="wo", bufs=1))
            psC = ph.enter_context(tc.tile_pool(name="psC", bufs=4, space="PSUM"))
            outC = ph.enter_context(tc.tile_pool(name="outC", bufs=2))

            wo_t = wo_pool.tile([P, 4, DQ], mm_dt, tag="wo", name="wo")
            nc.sync.dma_start(wo_t[:], WoT_r[:])

            for m in range(DQ // P):  # 8
                ot = outC.tile([P, LQ], f32, tag="ot", name="ot")
                for n in range(NQ):  # 4
                    ps = psC.tile([P, 512], f32, tag="psC", name="psC")
                    for kt in range(4):
                        nc.tensor.matmul(
                            ps[:],
                            lhsT=wo_t[:, kt, m * P:(m + 1) * P],
                            rhs=ctxT[kt][:, n * 512:(n + 1) * 512],
                            start=(kt == 0),
                            stop=(kt == 3),
                        )
                    nc.scalar.copy(ot[:, n * 512:(n + 1) * 512], ps[:])
                nc.sync.dma_start(outT[m * P:(m + 1) * P, :], ot[:])


def _build():
    import concourse.bacc as bacc
    import concourse.mybir as mybir
    import concourse.tile as tile

    f32 = mybir.dt.float32
    in_dt = mybir.dt.float32r if USE_F32R else f32
    nc = bacc.Bacc("TRN2", target_bir_lowering=False, debug=False)
    aps = {
        "qT": nc.dram_tensor("qT", [DQ, LQ], in_dt, kind="ExternalInput").ap(),
        "kvT": nc.dram_tensor("kvT", [DKV, LKV], in_dt, kind="ExternalInput").ap(),
        "WqT": nc.dram_tensor("WqT", [DQ, GH], in_dt, kind="ExternalInput").ap(),
        "WkT": nc.dram_tensor("WkT", [DKV, GH], in_dt, kind="ExternalInput").ap(),
        "WvT": nc.dram_tensor("WvT", [DKV, GH], in_dt, kind="ExternalInput").ap(),
        "WoT": nc.dram_tensor("WoT", [GH, DQ], in_dt, kind="ExternalInput").ap(),
        "outT": nc.dram_tensor("outT", [DQ, LQ], f32, kind="ExternalOutput").ap(),
    }
    with tile.TileContext(nc) as tc:
        _emit(tc, aps)
    nc.compile()
    return nc


def make_in_maps(q, kv, Wq, Wk, Wv, Wo):
    in_maps = []
    for c in range(NCORES):
        b, g = divmod(c, 2)
        hs = slice(g * GH, (g + 1) * GH)
        in_maps.append({
            "qT": np.ascontiguousarray(q[b].T),
            "kvT": np.ascontiguousarray(kv[b].T),
            "WqT": np.ascontiguousarray(Wq[hs, :].T),
            "WkT": np.ascontiguousarray(Wk[hs, :].T),
            "WvT": np.ascontiguousarray(Wv[hs, :].T),
            "WoT": np.ascontiguousarray(Wo[:, hs].T),
        })
    return in_maps


def kernel(q, kv, Wq, Wk, Wv, Wo, bo):
    global _COMPILED, last_exec_time_ns, last_profile
    from concourse.bass_utils import run_bass_kernel_spmd

    if _COMPILED is None:
        _COMPILED = _build()
    nc = _COMPILED

    q = np.asarray(q, np.float32)
    kv = np.asarray(kv, np.float32)
    Wq = np.asarray(Wq, np.float32)
    Wk = np.asarray(Wk, np.float32)
    Wv = np.asarray(Wv, np.float32)
    Wo = np.asarray(Wo, np.float32)
    bo = np.asarray(bo, np.float32)

    in_maps = make_in_maps(q, kv, Wq, Wk, Wv, Wo)
    res = run_bass_kernel_spmd(nc, in_maps, core_ids=list(range(NCORES)),
                               trace=TRACE)
    last_exec_time_ns = res.exec_time_ns
    last_profile = res.profile_json

    out = np.empty((B, LQ, DQ), np.float32)
    for b in range(B):
        acc = res.results[2 * b]["outT"] + res.results[2 * b + 1]["outT"]
        out[b] = acc.T + bo
    return out
